# revision 1
# baseline (speedup 1.0000x reference)
"""Trainium2 kernel for nn_LoRALinear (moe_routing).

Math: reference computes out = x @ W.T + einsum('bri,bro->bo', a, b) with
a = A_table[dom].reshape(B,R,IN), b = B_table[dom].reshape(B,R,OUT).
The einsum contracts i over `a` alone, so the LoRA term collapses to a
per-domain table:
    L[d, o] = sum_r (sum_i A_table[d].reshape(R,IN)[r,i]) * B_table[d].reshape(R,OUT)[r,o]
    out = x @ W.T + L[domain_id]

Device work per core (data-parallel over batch, 2048 rows/core):
    out_tile[mt] = x[mt] @ W.T + Lg[mt]
where Lg = L[domain_id] is gathered on the host (a 64x1024 table lookup)
and streamed alongside x. The dense matmul runs as 16 m-tiles x 8 k-chunks
x 2 n-halves of [128x128] @ [128x512] bf16 MMs with the x block stationary
(LDWEIGHTS overlaps in-flight MMs via the background weight buffer, so the
PE streams at the 512-cycle/MM peak). The LoRA add rides the PSUM->SBUF
drain as a DVE tensor_add, so no partial-row-group matmuls are needed.

Outputs are written bf16 (host upcasts) to halve the store traffic; input
loads go on the sync HWDGE queue and stores on the scalar queue so they
don't head-of-line block each other.
"""

import functools

import numpy as np

import concourse.mybir as mybir
import concourse.tile as tile
from concourse import bacc, bass_utils

B, D, R, ND = 16384, 1024, 8, 64
N_CORES = 8
BS = B // N_CORES            # 2048 batch rows per core
NK = 8                       # k chunks of 128
NMT = BS // 128              # 16 m-tiles per core
MTW = 2 * D                  # xaug cols per m-tile: 1024 x-chunks + 1024 Lg

# m-tiles per DMA block: small blocks first so compute starts early
X_PLAN = [[0, 1], [2, 3], [4, 5, 6, 7], [8, 9, 10, 11], [12, 13, 14, 15]]


@functools.lru_cache(maxsize=1)
def _build():
    nc = bacc.Bacc(None, target_bir_lowering=False, debug=False)
    bf16 = mybir.dt.bfloat16
    f32 = mybir.dt.float32
    xa = nc.dram_tensor("xa", [128, NMT * MTW], bf16, kind="ExternalInput")
    wa = nc.dram_tensor("wa", [128, NK * D], bf16, kind="ExternalInput")
    out = nc.dram_tensor("out", [128, NMT * D], bf16, kind="ExternalOutput")

    with tile.TileContext(nc) as tc:
        with (
            tc.tile_pool(name="w", bufs=1) as wpool,
            tc.tile_pool(name="x", bufs=1) as xpool,
            tc.tile_pool(name="o", bufs=2) as opool,
            tc.tile_pool(name="ps", bufs=4, space="PSUM") as pspool,
        ):
            # Warm the PE (HAM clock gate) with dummy matmuls while the
            # first DMAs stream in. Memset on DVE so it isn't gated on the
            # slower GpSimd preamble; dummy MMs land in a psum pool slot
            # that gets recycled (start=True clears it before any real use).
            scratch = wpool.tile([128, 512], bf16, tag="scratch")
            nc.vector.memset(scratch[:], 0.0)
            dps = pspool.tile([128, 2 * 512], f32, tag="ps")
            for i in range(14):
                nc.tensor.matmul(
                    dps[:, 0:512], scratch[:, 0:128], scratch[:],
                    start=(i == 0), stop=(i == 13),
                )

            wts = []
            xtiles = {}

            def dma_w(j):
                # 0.5 MB tiles (4 KB per-partition lines, still near line
                # rate) have ~half the completion latency of 1 MB tiles, so
                # the first k-chunks unblock the PE sooner; 2 KB lines would
                # run at ~half BW
                wt = wpool.tile([128, 2 * D], bf16, tag=f"w{j}")
                nc.sync.dma_start(wt[:], wa[:, j * 2 * D : (j + 1) * 2 * D])
                wts.append(wt)

            def dma_x(g):
                mts = X_PLAN[g]
                t = xpool.tile([128, len(mts) * MTW], bf16, tag=f"x{g}")
                nc.sync.dma_start(
                    t[:], xa[:, mts[0] * MTW : (mts[-1] + 1) * MTW]
                )
                for i, mt in enumerate(mts):
                    xtiles[mt] = (t, i * MTW)

            # issue order = consumption order: x block for m-tiles 0-1,
            # then the four W quarters (each unblocks two k-chunks), then
            # the rest of x
            dma_x(0)
            for j in range(4):
                dma_w(j)
            for g in range(1, len(X_PLAN)):
                dma_x(g)

            # Prologue: m-tiles 0-1 k-banded so each arriving W quarter
            # immediately feeds 8 matmuls — the PE stays saturated while
            # the W stream lands instead of idling then catching up.
            pro = {}
            for mt in range(2):
                xt, xof = xtiles[mt]
                ps = pspool.tile([128, 2 * 512], f32, tag="ps")
                pro[mt] = (ps, xt, xof)
            for kb in range(4):
                for mt in range(2):
                    ps, xt, xof = pro[mt]
                    for k in (2 * kb, 2 * kb + 1):
                        wt = wts[k // 2]
                        wof = (k % 2) * D
                        lhsT = xt[:, xof + k * 128 : xof + (k + 1) * 128]
                        nc.tensor.matmul(
                            ps[:, 0:512], lhsT, wt[:, wof : wof + 512],
                            start=(k == 0), stop=(k == NK - 1),
                        )
                        nc.tensor.matmul(
                            ps[:, 512:1024], lhsT, wt[:, wof + 512 : wof + D],
                            start=(k == 0), stop=(k == NK - 1),
                        )

            ot = None
            for mt in range(NMT):
                if mt < 2:
                    ps, xt, xof = pro[mt]
                else:
                    xt, xof = xtiles[mt]
                    ps = pspool.tile([128, 2 * 512], f32, tag="ps")
                last = mt == NMT - 1
                if mt < 2:
                    pass  # matmuls already issued in the prologue
                elif not last:
                    for k in range(NK):
                        wt = wts[k // 2]
                        wof = (k % 2) * D
                        lhsT = xt[:, xof + k * 128 : xof + (k + 1) * 128]
                        nc.tensor.matmul(
                            ps[:, 0:512], lhsT, wt[:, wof : wof + 512],
                            start=(k == 0), stop=(k == NK - 1),
                        )
                        nc.tensor.matmul(
                            ps[:, 512:1024], lhsT, wt[:, wof + 512 : wof + D],
                            start=(k == 0), stop=(k == NK - 1),
                        )
                else:
                    # final m-tile: run each n-half's k-chain to completion
                    # separately so its add + store pipeline with the other
                    # half's matmuls (shrinks the kernel drain tail)
                    for h in range(2):
                        for k in range(NK):
                            wt = wts[k // 2]
                            wof = (k % 2) * D + h * 512
                            lhsT = xt[:, xof + k * 128 : xof + (k + 1) * 128]
                            nc.tensor.matmul(
                                ps[:, h * 512 : (h + 1) * 512],
                                lhsT,
                                wt[:, wof : wof + 512],
                                start=(k == 0), stop=(k == NK - 1),
                            )
                if mt % 2 == 0:
                    ot = opool.tile([128, 2 * D], bf16, tag="ot")
                oof = (mt % 2) * D
                if last:
                    for h in range(2):
                        nc.vector.tensor_add(
                            ot[:, oof + h * 512 : oof + (h + 1) * 512],
                            ps[:, h * 512 : (h + 1) * 512],
                            xt[:, xof + D + h * 512 : xof + D + (h + 1) * 512],
                        )
                        nc.scalar.dma_start(
                            out[:, mt * D + h * 512 : mt * D + (h + 1) * 512],
                            ot[:, oof + h * 512 : oof + (h + 1) * 512],
                        )
                else:
                    nc.vector.tensor_add(
                        ot[:, oof : oof + D],
                        ps[:],
                        xt[:, xof + D : xof + 2 * D],
                    )
                    if mt == NMT - 2:
                        nc.scalar.dma_start(
                            out[:, mt * D : (mt + 1) * D], ot[:, oof : oof + D]
                        )
                    elif mt % 2 == 1:
                        nc.scalar.dma_start(
                            out[:, (mt - 1) * D : (mt + 1) * D], ot[:]
                        )

    nc.compile()
    return nc


def _prepare(x, W, A_table, B_table, domain_id):
    import ml_dtypes

    bf16 = np.dtype(ml_dtypes.bfloat16)
    x = np.asarray(x, dtype=np.float32)
    W = np.asarray(W, dtype=np.float32)
    A = np.asarray(A_table, dtype=np.float64)
    Bt = np.asarray(B_table, dtype=np.float64)
    dom = np.asarray(domain_id).astype(np.int64)

    sA = A.reshape(ND, R, D).sum(axis=2)                        # [ND, R]
    L = np.einsum("dr,dro->do", sA, Bt.reshape(ND, R, D))       # [ND, D]
    Lg = L.astype(np.float32)[dom].astype(bf16)                 # [B, D]

    # W.T chunk-major: wa[p, k*D + n] = W.T[k*128+p, n]
    wa = np.ascontiguousarray(
        W.T.astype(bf16).reshape(NK, 128, D).transpose(1, 0, 2)
    ).reshape(128, NK * D)

    in_maps = []
    for c in range(N_CORES):
        sl = slice(c * BS, (c + 1) * BS)
        xc = x[sl].astype(bf16)                                 # [2048, 1024]
        # xpart[p, mt, k*128+j] = xc[mt*128+j, k*128+p]
        xpart = xc.reshape(NMT, 128, NK, 128).transpose(3, 0, 2, 1)
        lgpart = Lg[sl].reshape(NMT, 128, D).transpose(1, 0, 2)  # [p, mt, n]
        xaug = np.empty((128, NMT, MTW), dtype=bf16)
        xaug[:, :, 0:D] = xpart.reshape(128, NMT, D)
        xaug[:, :, D:MTW] = lgpart
        in_maps.append({"xa": xaug.reshape(128, NMT * MTW), "wa": wa})
    return in_maps


def kernel(x, W, A_table, B_table, domain_id, _trace=False):
    in_maps = _prepare(x, W, A_table, B_table, domain_id)
    nc = _build()
    res = bass_utils.run_bass_kernel_spmd(
        nc, in_maps, core_ids=list(range(N_CORES)), trace=_trace
    )
    outs = []
    for c in range(N_CORES):
        oc = res.results[c]["out"]                              # [128, NMT*D] bf16
        outs.append(
            oc.reshape(128, NMT, D)
            .transpose(1, 0, 2)
            .reshape(BS, D)
            .astype(np.float32)
        )
    out = np.concatenate(outs, axis=0)
    if _trace:
        kernel.last_results = res
    return out



# revision 2
# speedup vs baseline: 1.0041x; 1.0041x over previous
"""Trainium2 kernel for nn_LoRALinear (moe_routing), schedule v6.

out = x @ W.T + L[domain_id]; L collapses the LoRA einsum to a 64x1024
per-domain table computed on host.

Dtypes: x fp8-e3m4 (lossless upcast in the PE, halves x traffic), W bf16
(its scale lives in e3m4's subnormal zone), Lg fp8-e4m3, out bf16.
~6MB in + 4MB out per core; rel err ~1.2e-2 (gate 2e-2).

Schedule notes (from trace archaeology):
  - Engines reach the kernel's first instruction ~7us (framework
    preamble); DMA doorbell->first byte ~1.5us; 16-SDMA stagger ~1us.
  - Both HWDGE rings share the 16 SDMA engines packet-round-robin, so
    "parallel" rings halve each other's bandwidth; order packets by
    consumption deadline instead and keep late data paced.
  - Tile dep tracking is tile-granular: the last m-tile's halves use
    their own PSUM tiles so their adds overlap the remaining matmuls.
  - HAM clock gate: ~3.4us of sustained PE activity before 2.4GHz;
    dummy matmuls fill the DMA wait so real matmuls start warm.
"""

import functools

import numpy as np

import concourse.mybir as mybir
import concourse.tile as tile
from concourse import bacc, bass_utils

B, D, R, ND = 16384, 1024, 8, 64
N_CORES = 8
BS = B // N_CORES            # 2048 batch rows per core
NK = 8                       # k chunks of 128
NMT = BS // 128              # 16 m-tiles per core

N_DUMMY = 9                  # warmup matmuls (HAM un-throttle)

W_PLAN = [[0], [1], [2], [3], [4, 5], [6, 7]]
X_PACED = [[2, 3], [4, 5], [6, 7], [8, 9], [10, 11], [12, 13], [14, 15]]


@functools.lru_cache(maxsize=1)
def _build():
    nc = bacc.Bacc(None, target_bir_lowering=False, debug=False)
    bf16 = mybir.dt.bfloat16
    e3 = mybir.dt.float8e3
    e4 = mybir.dt.float8e4
    f32 = mybir.dt.float32
    xa = nc.dram_tensor("xa", [128, NMT * D], e3, kind="ExternalInput")
    lg = nc.dram_tensor("lg", [128, NMT * D], e4, kind="ExternalInput")
    wa = nc.dram_tensor("wa", [128, NK * D], bf16, kind="ExternalInput")
    out = nc.dram_tensor("out", [128, NMT * D], bf16, kind="ExternalOutput")

    with tile.TileContext(nc) as tc:
        with (
            tc.tile_pool(name="w", bufs=1) as wpool,
            tc.tile_pool(name="x", bufs=2) as xpool,
            tc.tile_pool(name="lg", bufs=1) as lgpool,
            tc.tile_pool(name="o", bufs=2) as opool,
            tc.tile_pool(name="ps", bufs=3, space="PSUM") as pspool,
            tc.tile_pool(name="psq", bufs=2, space="PSUM") as psqpool,
        ):
            xtiles = {}

            def dma_x1(mt, eng):
                t = xpool.tile([128, D], e3, tag=f"xs{mt}")
                eng.dma_start(t[:], xa[:, mt * D : (mt + 1) * D])
                xtiles[mt] = (t, 0)

            def dma_x2(g, eng):
                mts = X_PACED[g]
                t = xpool.tile([128, 2 * D], e3, tag="xb")
                eng.dma_start(t[:], xa[:, mts[0] * D : (mts[-1] + 1) * D])
                for i, mt in enumerate(mts):
                    xtiles[mt] = (t, i * D)

            wts = {}

            def dma_w(g):
                ks = W_PLAN[g]
                t = wpool.tile([128, len(ks) * D], bf16, tag=f"w{g}")
                nc.scalar.dma_start(t[:], wa[:, ks[0] * D : (ks[-1] + 1) * D])
                for i, k in enumerate(ks):
                    wts[k] = (t, i * D)

            # sync ring: x0 | x1 | x23 | x45 | paced x67..x1415
            # scalar ring: Wk0 | Wk1 | Wk23 | Wk45 | Wk67 | lg0-3 | lg4-15,
            # then the 2-m-tile pair stores
            dma_x1(0, nc.sync)
            dma_w(0)
            dma_x1(1, nc.sync)
            for g in range(1, len(W_PLAN)):
                dma_w(g)
            dma_x2(0, nc.sync)
            lgt0 = lgpool.tile([128, 4 * D], e4, tag="lg0")
            nc.scalar.dma_start(lgt0[:], lg[:, 0 : 4 * D])
            dma_x2(1, nc.scalar)
            lgt1 = lgpool.tile([128, 12 * D], e4, tag="lg1")
            nc.scalar.dma_start(lgt1[:], lg[:, 4 * D : 16 * D])
            for g in range(2, len(X_PACED)):
                dma_x2(g, nc.sync)

            def lgs(mt):
                if mt < 4:
                    return lgt0[:, mt * D : (mt + 1) * D]
                return lgt1[:, (mt - 4) * D : (mt - 3) * D]

            # warmup dummies: sustained PE activity through the DMA wait so
            # the HAM gate is at 2.4GHz when the real stream starts
            scratch = wpool.tile([128, 640], bf16, tag="scratch")
            nc.vector.memset(scratch[:], 0.0)
            dps = pspool.tile([128, D], f32, tag="ps")
            for i in range(N_DUMMY):
                nc.tensor.matmul(
                    dps[:, 0:512], scratch[:, 512:640], scratch[:, 0:512],
                    start=(i == 0), stop=(i == N_DUMMY - 1),
                )

            # prologue: mt0 k0 alone (x1 lands just after x0), then mt0+mt1
            # banded over the arriving W chunks
            pro = {}
            for mt in range(2):
                ps = pspool.tile([128, D], f32, tag="ps")
                pro[mt] = ps

            def mm_pair(ps, xt, xof, k, start, stop):
                wt, wof = wts[k]
                lhsT = xt[:, xof + k * 128 : xof + (k + 1) * 128]
                nc.tensor.matmul(
                    ps[:, 0:512], lhsT, wt[:, wof : wof + 512],
                    start=start, stop=stop,
                )
                nc.tensor.matmul(
                    ps[:, 512:1024], lhsT, wt[:, wof + 512 : wof + D],
                    start=start, stop=stop,
                )

            mm_pair(pro[0], *xtiles[0], 0, True, False)
            mm_pair(pro[1], *xtiles[1], 0, True, False)
            for k in range(1, NK):
                for mt in range(2):
                    mm_pair(pro[mt], *xtiles[mt], k, False, k == NK - 1)

            ot = None
            for mt in range(NMT):
                last = mt == NMT - 1
                if mt < 2:
                    ps = pro[mt]
                elif not last:
                    xt, xof = xtiles[mt]
                    ps = pspool.tile([128, D], f32, tag="ps")
                    for k in range(NK):
                        mm_pair(ps, xt, xof, k, k == 0, k == NK - 1)

                if last:
                    # final m-tile: quarter chains into alternating PSUM
                    # banks (own tiles - dep tracking is tile-granular) so
                    # each quarter's add and store overlap the remaining
                    # matmuls; stores ride the idle sync ring
                    xt, xof = xtiles[mt]
                    ote = opool.tile([128, D], bf16, tag="ot15")
                    for q in range(4):
                        psh = psqpool.tile([128, 256], f32, tag="psq")
                        for k in range(NK):
                            wt, wof = wts[k]
                            lhsT = xt[:, xof + k * 128 : xof + (k + 1) * 128]
                            nc.tensor.matmul(
                                psh[:], lhsT,
                                wt[:, wof + q * 256 : wof + q * 256 + 256],
                                start=(k == 0), stop=(k == NK - 1),
                            )
                        sl = slice(q * 256, (q + 1) * 256)
                        nc.vector.tensor_add(ote[:, sl], psh[:], lgs(mt)[:, sl])
                        nc.sync.dma_start(
                            out[:, mt * D + q * 256 : mt * D + (q + 1) * 256],
                            ote[:, sl],
                        )
                elif mt == NMT - 2:
                    ote = opool.tile([128, D], bf16, tag="ot14")
                    nc.vector.tensor_add(ote[:], ps[:], lgs(mt))
                    nc.sync.dma_start(out[:, mt * D : (mt + 1) * D], ote[:])
                else:
                    if mt % 2 == 0:
                        ot = opool.tile([128, 2 * D], bf16, tag="ot")
                    oof = (mt % 2) * D
                    nc.vector.tensor_add(ot[:, oof : oof + D], ps[:], lgs(mt))
                    if mt % 2 == 1:
                        nc.scalar.dma_start(
                            out[:, (mt - 1) * D : (mt + 1) * D], ot[:]
                        )

    nc.compile()
    return nc


def _prepare(x, W, A_table, B_table, domain_id):
    import ml_dtypes

    bf16 = np.dtype(ml_dtypes.bfloat16)
    e3 = np.dtype(ml_dtypes.float8_e3m4)
    e4 = np.dtype(ml_dtypes.float8_e4m3)
    x = np.asarray(x, dtype=np.float32)
    W = np.asarray(W, dtype=np.float32)
    A = np.asarray(A_table, dtype=np.float64)
    Bt = np.asarray(B_table, dtype=np.float64)
    dom = np.asarray(domain_id).astype(np.int64)

    sA = A.reshape(ND, R, D).sum(axis=2)                        # [ND, R]
    L = np.einsum("dr,dro->do", sA, Bt.reshape(ND, R, D))       # [ND, D]
    Lg = L.astype(np.float32)[dom].astype(e4)                   # [B, D]

    # W.T chunk-major: wa[p, k*D + n] = W.T[k*128+p, n]
    wa = np.ascontiguousarray(
        W.T.astype(bf16).reshape(NK, 128, D).transpose(1, 0, 2)
    ).reshape(128, NK * D)

    x8 = x.astype(e3)
    in_maps = []
    for c in range(N_CORES):
        sl = slice(c * BS, (c + 1) * BS)
        xc = x8[sl]                                             # [2048, 1024] e3
        # xpart[p, mt, k*128+j] = xc[mt*128+j, k*128+p]
        xpart = np.ascontiguousarray(
            xc.reshape(NMT, 128, NK, 128).transpose(3, 0, 2, 1)
        ).reshape(128, NMT * D)
        lgpart = np.ascontiguousarray(
            Lg[sl].reshape(NMT, 128, D).transpose(1, 0, 2)
        ).reshape(128, NMT * D)
        in_maps.append({"xa": xpart, "lg": lgpart, "wa": wa})
    return in_maps


def kernel(x, W, A_table, B_table, domain_id, _trace=False):
    in_maps = _prepare(x, W, A_table, B_table, domain_id)
    nc = _build()
    res = bass_utils.run_bass_kernel_spmd(
        nc, in_maps, core_ids=list(range(N_CORES)), trace=_trace
    )
    outs = []
    for c in range(N_CORES):
        oc = res.results[c]["out"]                              # [128, NMT*D] bf16
        outs.append(
            oc.reshape(128, NMT, D)
            .transpose(1, 0, 2)
            .reshape(BS, D)
            .astype(np.float32)
        )
    out = np.concatenate(outs, axis=0)
    if _trace:
        kernel.last_results = res
    return out
